# revision 1
# baseline (speedup 1.0000x reference)
"""Trainium2 Bass kernel for nn_ContrastiveLoss (exp-cosine ranking loss).

Math: sort rows of output1 by descending ranking (stable). With
e_b[i] = exp(cos_sim(x_sorted[i], o_b)) for b in {2,3} and suffix sums
suf_b(i) = sum_{j>=i} e_b[j], the reference loss equals

    loss = N*(log T2 + log T3) - sum_i log suf2(i) - sum_i log suf3(i)

where T_b = suf_b(0) is the global total.  Sharding: host sorts by
ranking (shards are rank-contiguous) and feeds rows in ASCENDING rank
order so forward cumsums on-device are exactly the suffix sums of the
reference order.  Each core gets its 8192-row shard TRANSPOSED
[512, 8192] in bf16; o2/o3 are pre-normalized on the host so no norm
prep or reciprocal is needed on device.

Per 512-row block (16 blocks/core), everything is *streamed* through
the PE array (no transposes, no 128-col weight loads):

  dots:  normalized o23 pair stationary at columns (2j, 2j+1),
         xT chunk [128,512] moving -> row dots land directly on the
         block's own PSUM partitions, accumulated across an 8-block
         half-shard into a [16,512] PSUM tile
  norms: DVE squares the xT tile (bf16, 2 elem/cycle); ones placed the
         same way -> second [16,512] PSUM tile = row |x|^2
  1/|x|: ACT exp(-0.5*ln(ssq)) (avoids the slow DVE reciprocal)

Bulk DMA alternates between the two HWDGE queues (sync + scalar),
keeping the gpsimd queue free for the one tiny AllGather.  The shard is
split into two halves so half A's exp/totals run in the shadow of half
B's streaming and the AllGather posts right after the last matmul; the
per-block scans (seeded by strict-lower-triangular matmuls) and the Ln
table preload hide in the AllGather wait.  Each core outputs
(tot2, tot3, sum-of-logs); the host sums 8 of each and forms
N*(log T2 + log T3) - sum(partials).
"""

import numpy as np

N, D = 65536, 512
NCORES = 8
SH = N // NCORES            # 8192 rows per core
NCH = D // 128              # 4 contraction chunks of 128
RBLK = 512                  # rows per block
NBLK = SH // RBLK           # 16 blocks
HB = NBLK // 2              # 8 blocks per half-shard
NP16 = 2 * HB               # 16 stats partitions per half: (j, b) -> 2j+b

_compiled_nc = None


def _half_tail(nc, mybir, sp, dots_ps, ssq_ps, e16, tot16):
    """ssq/dots [16,512] PSUM -> e = exp(cos) [16,512] and totals [16,1]."""
    AF = mybir.ActivationFunctionType
    OP = mybir.AluOpType
    AX = mybir.AxisListType
    ls = sp.tile([NP16, RBLK], mybir.dt.float32, tag="ls")
    nc.scalar.activation(ls[:], ssq_ps[:], AF.Ln)
    rs = sp.tile([NP16, RBLK], mybir.dt.float32, tag="rs")
    nc.scalar.activation(rs[:], ls[:], AF.Exp, scale=-0.5)
    t16 = sp.tile([NP16, RBLK], mybir.dt.float32, tag="t16")
    nc.vector.tensor_tensor(out=t16[:], in0=dots_ps[:], in1=rs[:], op=OP.mult)
    nc.scalar.activation(e16[:], t16[:], AF.Exp)
    nc.vector.tensor_reduce(out=tot16[:], in_=e16[:], axis=AX.X, op=OP.add)


def _body(tc, mybir, xs, o23w_d, onesw_d, la_d, ma_d, mlt16_d, sel16_d,
          fin_out):
    nc = tc.nc
    f32 = mybir.dt.float32
    bf16 = mybir.dt.bfloat16
    OP = mybir.AluOpType
    AF = mybir.ActivationFunctionType

    with (
        tc.tile_pool(name="const", bufs=1) as constp,
        tc.tile_pool(name="xin", bufs=6) as xinp,
        tc.tile_pool(name="sq", bufs=3) as sqp,
        tc.tile_pool(name="stats", bufs=1) as statsp,
        tc.tile_pool(name="scr", bufs=2) as scrp,
        tc.tile_pool(name="small", bufs=1) as smallp,
        tc.tile_pool(name="psum", bufs=1, space="PSUM") as psump,
        tc.tile_pool(name="dram", bufs=1, space="DRAM") as dramp,
    ):
        # ---- PE warm-up: pull the HAM clock gate to 8/8 before the real
        # stream arrives (first xt DMA lands ~10us in)
        wsrc = constp.tile([128, RBLK], bf16)
        nc.vector.memset(wsrc[:], 0.0)
        warm_ps = psump.tile([NP16, RBLK], f32, tag="warm", bufs=1)
        for _ in range(12):
            nc.tensor.matmul(warm_ps[:], wsrc[:, 0:NP16], wsrc[:],
                             start=True, stop=True)

        # ---- constants (small, on the gpsimd queue) ----
        o23w = constp.tile([128, NBLK, NCH, NP16], bf16)
        nc.gpsimd.dma_start(o23w[:], o23w_d)
        onesw = constp.tile([128, NBLK, NP16], bf16)
        nc.gpsimd.dma_start(onesw[:], onesw_d)
        la = constp.tile([NP16, NP16], f32)
        nc.gpsimd.dma_start(la[:], la_d)
        ma = constp.tile([NP16, NP16], f32)
        nc.gpsimd.dma_start(ma[:], ma_d)
        mlt16 = constp.tile([2 * NCORES, NP16], f32)
        nc.gpsimd.dma_start(mlt16[:], mlt16_d)
        sel16 = constp.tile([NP16, 2], f32)
        nc.gpsimd.dma_start(sel16[:], sel16_d)
        ones16 = constp.tile([NP16, 1], f32)
        nc.vector.memset(ones16[:], 1.0)

        # dummy AllGather: pays the CC stream's first-op sync/setup cost
        # in the shadow of the main loop so the real one starts promptly
        cc0_in = dramp.tile([1, 1], f32)
        cc0_out = dramp.tile([NCORES, 1], f32, addr_space="Shared")
        nc.sync.dma_start(cc0_in[:], ones16[0:1, :])
        nc.gpsimd.collective_compute(
            "AllGather", OP.bypass, replica_groups=[list(range(NCORES))],
            ins=[cc0_in.opt()], outs=[cc0_out.opt()])

        # ---- streamed main loop over two half-shards ----
        dotsA = psump.tile([NP16, RBLK], f32, tag="dotsA", bufs=1)
        ssqA = psump.tile([NP16, RBLK], f32, tag="ssqA", bufs=1)
        dotsB = psump.tile([NP16, RBLK], f32, tag="dotsB", bufs=1)
        ssqB = psump.tile([NP16, RBLK], f32, tag="ssqB", bufs=1)
        eA = statsp.tile([NP16, RBLK], f32)
        totA = smallp.tile([NP16, 1], f32)
        eB = statsp.tile([NP16, RBLK], f32)
        totB = smallp.tile([NP16, 1], f32)

        # xs is xT [D, SH]; tile (p=d-in-chunk, c=chunk, r=row-in-block)
        xv = xs.rearrange("(c p) (g r) -> g p c r", p=128, g=NBLK)
        for g in range(NBLK):
            j = g % HB
            dots_ps, ssq_ps = (dotsA, ssqA) if g < HB else (dotsB, ssqB)
            xt = xinp.tile([128, NCH, RBLK], bf16)
            if g % 2 == 0:
                nc.sync.dma_start(xt[:], xv[g])
            else:
                nc.scalar.dma_start(xt[:], xv[g])
            for c in range(NCH):
                nc.tensor.matmul(
                    dots_ps[:], o23w[:, g, c, :], xt[:, c, :],
                    start=(j == 0 and c == 0),
                    stop=(j == HB - 1 and c == NCH - 1))
            sq = sqp.tile([128, NCH, RBLK], bf16)
            nc.vector.tensor_tensor(out=sq[:], in0=xt[:], in1=xt[:],
                                    op=OP.mult)
            for c in range(NCH):
                nc.tensor.matmul(
                    ssq_ps[:], onesw[:, g, :], sq[:, c, :],
                    start=(j == 0 and c == 0),
                    stop=(j == HB - 1 and c == NCH - 1))
            if g == HB - 1:
                # half A's exp-cosine tail overlaps half B's streaming
                _half_tail(nc, mybir, scrp, dotsA, ssqA, eA, totA)
                exclA_ps = psump.tile([NP16, 1], f32, tag="tail", bufs=2)
                nc.tensor.matmul(exclA_ps[:], la[:], totA[:], start=True,
                                 stop=True)
                basecA = smallp.tile([NP16, 1], f32)
                nc.vector.tensor_copy(basecA[:], exclA_ps[:])
                sufA = statsp.tile([NP16, RBLK], f32)
                nc.vector.tensor_tensor_scan(
                    out=sufA[:], data0=eA[:], data1=eA[:], initial=basecA[:],
                    op0=OP.add, op1=OP.bypass)

        _half_tail(nc, mybir, scrp, dotsB, ssqB, eB, totB)

        # core totals [2,1] -> AllGather, posted as early as possible
        tl_ps = psump.tile([2, 1], f32, tag="tail", bufs=2)
        nc.tensor.matmul(tl_ps[:], sel16[:], totA[:], start=True, stop=False)
        nc.tensor.matmul(tl_ps[:], sel16[:], totB[:], start=False, stop=True)
        tl = smallp.tile([2, 1], f32)
        nc.vector.tensor_copy(tl[:], tl_ps[:])
        cc_in = dramp.tile([2, 1], f32)
        cc_out = dramp.tile([2 * NCORES, 1], f32, addr_space="Shared")
        nc.sync.dma_start(cc_in[:], tl[:])
        nc.gpsimd.collective_compute(
            "AllGather", OP.bypass, replica_groups=[list(range(NCORES))],
            ins=[cc_in.opt()], outs=[cc_out.opt()])

        # overlap the AllGather wait: half-B bases + scan, Ln table preload
        exclB_ps = psump.tile([NP16, 1], f32, tag="tail", bufs=2)
        nc.tensor.matmul(exclB_ps[:], ma[:], totA[:], start=True, stop=False)
        nc.tensor.matmul(exclB_ps[:], la[:], totB[:], start=False, stop=True)
        basecB = smallp.tile([NP16, 1], f32)
        nc.vector.tensor_copy(basecB[:], exclB_ps[:])
        sufB = statsp.tile([NP16, RBLK], f32)
        nc.vector.tensor_tensor_scan(
            out=sufB[:], data0=eB[:], data1=eB[:], initial=basecB[:],
            op0=OP.add, op1=OP.bypass)
        lnwarm = smallp.tile([NP16, 1], f32)
        nc.scalar.activation(lnwarm[:], totA[:], AF.Ln)

        # consume the AllGather: per-partition cross-core bases
        ag = smallp.tile([2 * NCORES, 1], f32)
        nc.sync.dma_start(ag[:], cc_out[:])
        gb_ps = psump.tile([NP16, 1], f32, tag="tail", bufs=2)
        nc.tensor.matmul(gb_ps[:], mlt16[:], ag[:], start=True, stop=True)
        gb16 = smallp.tile([NP16, 1], f32)
        nc.vector.tensor_copy(gb16[:], gb_ps[:])

        # log-reduction (cross-core base folded into the Ln bias)
        lnA = scrp.tile([NP16, RBLK], f32, tag="ls")
        laA = smallp.tile([NP16, 1], f32)
        nc.scalar.activation(lnA[:], sufA[:], AF.Ln, bias=gb16[:],
                             accum_out=laA[:])
        lnB = scrp.tile([NP16, RBLK], f32, tag="ls")
        laB = smallp.tile([NP16, 1], f32)
        nc.scalar.activation(lnB[:], sufB[:], AF.Ln, bias=gb16[:],
                             accum_out=laB[:])
        part_ps = psump.tile([1, 1], f32, tag="tail", bufs=2)
        nc.tensor.matmul(part_ps[:], ones16[:], laA[:], start=True, stop=False)
        nc.tensor.matmul(part_ps[:], ones16[:], laB[:], start=False, stop=True)

        # per-core outputs: fin[0,0]=tot2, fin[1,0]=tot3, fin[0,1]=partial
        finsb = smallp.tile([2, 2], f32)
        nc.vector.tensor_copy(finsb[:, 0:1], tl[:])
        nc.vector.tensor_copy(finsb[0:1, 1:2], part_ps[:])
        nc.sync.dma_start(fin_out[:], finsb[:])


def build_nc():
    global _compiled_nc
    if _compiled_nc is not None:
        return _compiled_nc
    import concourse.bacc as bacc
    import concourse.mybir as mybir
    from concourse import tile

    f32 = mybir.dt.float32
    bf16 = mybir.dt.bfloat16
    nc = bacc.Bacc("TRN2", target_bir_lowering=False, debug=False,
                   num_devices=NCORES)
    xs = nc.dram_tensor("xs", [D, SH], bf16, kind="ExternalInput")
    o23w = nc.dram_tensor("o23w", [128, NBLK, NCH, NP16], bf16,
                          kind="ExternalInput")
    onesw = nc.dram_tensor("onesw", [128, NBLK, NP16], bf16,
                           kind="ExternalInput")
    la = nc.dram_tensor("la", [NP16, NP16], f32, kind="ExternalInput")
    ma = nc.dram_tensor("ma", [NP16, NP16], f32, kind="ExternalInput")
    mlt16 = nc.dram_tensor("mlt16", [2 * NCORES, NP16], f32,
                           kind="ExternalInput")
    sel16 = nc.dram_tensor("sel16", [NP16, 2], f32, kind="ExternalInput")
    fin = nc.dram_tensor("fin", [2, 2], f32, kind="ExternalOutput")

    with tile.TileContext(nc) as tc:
        _body(tc, mybir, xs.ap(), o23w.ap(), onesw.ap(), la.ap(), ma.ap(),
              mlt16.ap(), sel16.ap(), fin.ap())
    nc.compile()
    _compiled_nc = nc
    return nc


def make_in_maps(output1, output2, output3, ranking):
    """Host-side shard: stable sort by descending ranking (matching
    jnp.argsort(-ranking)), feed rows in ascending-rank order so forward
    cumsums on-device are the reference's suffix sums; transposed [D, SH]
    bf16 layout per shard; o2/o3 pre-normalized."""
    import ml_dtypes
    bf = ml_dtypes.bfloat16
    ranking = np.asarray(ranking, dtype=np.float32)
    order = np.argsort(-ranking, kind="stable")
    rho = order[::-1]
    xs_full = np.asarray(output1, dtype=np.float32)[rho].astype(bf)
    o2 = np.asarray(output2, dtype=np.float32).reshape(D)
    o3 = np.asarray(output3, dtype=np.float32).reshape(D)
    o2 = o2 / np.linalg.norm(o2)
    o3 = o3 / np.linalg.norm(o3)
    o23 = np.empty((128, NCH, 2), np.float32)
    o23[:, :, 0] = o2.reshape(NCH, 128).T
    o23[:, :, 1] = o3.reshape(NCH, 128).T
    # per-block stationaries: block g's (o2,o3) pair sits at columns
    # (2j, 2j+1), j = g mod 8, so its matmuls write its own PSUM partitions
    o23w = np.zeros((128, NBLK, NCH, NP16), np.float32)
    onesw = np.zeros((128, NBLK, NP16), np.float32)
    for g in range(NBLK):
        j = g % HB
        o23w[:, g, :, 2 * j : 2 * j + 2] = o23
        onesw[:, g, 2 * j : 2 * j + 2] = 1.0
    o23w = o23w.astype(bf)
    onesw = onesw.astype(bf)
    pidx = np.arange(NP16)
    par_match = pidx[:, None] % 2 == pidx[None, :] % 2
    la = ((pidx[:, None] < pidx[None, :]) & par_match).astype(np.float32)
    ma = par_match.astype(np.float32)
    sel16 = np.stack([(pidx % 2 == 0), (pidx % 2 == 1)], axis=1)
    sel16 = sel16.astype(np.float32)
    in_maps = []
    for c in range(NCORES):
        row = np.arange(2 * NCORES)
        mlt16 = ((row[:, None] // 2 < c)
                 & (row[:, None] % 2 == pidx[None, :] % 2)).astype(np.float32)
        in_maps.append({
            "xs": np.ascontiguousarray(xs_full[c * SH : (c + 1) * SH].T),
            "o23w": o23w, "onesw": onesw, "la": la, "ma": ma,
            "mlt16": mlt16, "sel16": sel16,
        })
    return in_maps


def kernel(output1, output2, output3, ranking):
    from concourse.bass_utils import run_bass_kernel_spmd

    nc = build_nc()
    in_maps = make_in_maps(output1, output2, output3, ranking)
    res = run_bass_kernel_spmd(nc, in_maps, core_ids=list(range(NCORES)))
    fins = [np.asarray(r["fin"], dtype=np.float64) for r in res.results]
    t2 = sum(f[0, 0] for f in fins)
    t3 = sum(f[1, 0] for f in fins)
    parts = sum(f[0, 1] for f in fins)
    loss = N * (np.log(t2) + np.log(t3)) - parts
    return np.asarray(loss, dtype=np.float32).reshape(())

